# revision 31
# baseline (speedup 1.0000x reference)
"""ContMix kernel for TRN2, 8 NeuronCores — v3.

Sharding: (batch b, H-half) -> 8 cores. Each core computes out[b, :, r0:r0+28, :].

v3 pipeline (per core):
  PE warm-up matmuls (ident@ident) ramp the PE clock to full speed early.
  pooling on PE, c-major: lhsT = ctx_t chunk [112, 128c], rhs = ind [112, 7]
    -> po3 [128, u, br, bin] f32 accumulated 4 mms per (u, br); one DVE
    packing copy -> ctx_p [128, 3, 49]  (no transpose roundtrip)
  kf = Wk @ ctx_p ; G = Wq^T @ kf  (f16 matmuls)
  B1 pair-granular (112 cols): logits lg_p = G^T xn_p; exp; dyn_p = expa_p^T
    wwdt1; normalize -> d16a   (software-pipelined across engines)
  scatter calls of [1,2,2,2,2,2,2,1] pairs -> M^T; PE transpose -> M chunks
  finals: banded matmuls xt^T @ M per scatter-call group; out in 4 DMAs
DMA order: blob(+ind), sidx, ctx_t(8pc), xn(3pc), xt(4pc) — all f16.
"""

import numpy as np

B, C, H, W = 4, 384, 56, 56
KK, S = 5, 7
NCORES = 8
ROWS = H // 2              # 28 rows per core
NPIX = ROWS * W            # 1568
PADR = ROWS + 4            # 32 padded rows
PADW = 64                  # padded width
WSPACE = PADR * PADW       # 2048 padded pixels
NPAIR = ROWS // 2          # 14 output row-pairs
D2 = C // 2                # 192
NI = 26                    # scatter idxs (25 taps + 1 denom col)
MCOLS = 3 * 128            # 384 = w''-space per pair
HW = H * W                 # 3136
NPP = HW // 2              # 1568 pixel-pairs (full image, for pooling)
FB = 1569                  # blob: wqA 0:384, wqB 384:768, wkA 768:1152, wkB 1152:1536, wwdt1 1536:1562, ind 1562:1569
NWARM = 24                 # PE clock warm-up matmuls

# scatter-call pair grouping: starts early (1 pair), ends small (1-pair calls)
GROUPS = [(0, 1), (1, 3), (3, 5), (5, 7), (7, 9), (9, 11), (11, 12), (12, 13), (13, 14)]
# out DMA pieces fired at end of finals(g): g -> (col0, col1)
OUTDMAS = {2: (0, 560), 4: (560, 1008), 5: (1008, 1232), 6: (1232, 1344),
           7: (1344, 1456), 8: (1456, 1568)}

_cached = {}


def _build_nc():
    import concourse.tile as tile
    from concourse import bacc, mybir, library_config, masks

    f32, f16, i16 = mybir.dt.float32, mybir.dt.float16, mybir.dt.int16
    nc = bacc.Bacc("TRN2", target_bir_lowering=False, debug=False)

    cxt_d = nc.dram_tensor("cxt", [NPP, 2 * C], f16, kind="ExternalInput")
    blob_d = nc.dram_tensor("blob", [128, FB], f16, kind="ExternalInput")
    xn_d = nc.dram_tensor("xn", [C, NPIX], f16, kind="ExternalInput")
    sidx_d = nc.dram_tensor("sidx", [2 * W, 2 * NI], i16, kind="ExternalInput")
    xt_d = nc.dram_tensor("xt", [WSPACE, C], f16, kind="ExternalInput")
    out_d = nc.dram_tensor("out", [C, NPIX], f16, kind="ExternalOutput")

    with tile.TileContext(nc) as tc:
        with (
            tc.tile_pool(name="big", bufs=1) as big,
            tc.tile_pool(name="wrk", bufs=3) as wrk,
            tc.tile_pool(name="mtp", bufs=3) as mtp,
            tc.tile_pool(name="ps", bufs=8, space="PSUM") as ps,
        ):
            # ---------------- input DMAs (SP queue = transfer order) --------------
            blob_sb = big.tile([128, FB], f16, tag="blob")
            nc.sync.dma_start(out=blob_sb[:], in_=blob_d[:])
            ind_sb = blob_sb[0:112, 1562:1569]
            sidx_sb = big.tile([2 * W, 2 * NI], i16, tag="sidx")
            nc.sync.dma_start(out=sidx_sb[:], in_=sidx_d[:])
            # ctx_t pieces: chunk t = 112 pixel-pairs = 4 image rows; bin-row br
            # = chunks 2br,2br+1.
            cxt_sb = big.tile([112, 14, 2 * C], f16, tag="cxt")
            CXP = [(0, 2), (2, 4), (4, 6), (6, 8), (8, 10), (10, 12), (12, 13), (13, 14)]
            for t0, t1 in CXP:
                nc.sync.dma_start(
                    out=cxt_sb[:, t0:t1, :],
                    in_=cxt_d[t0 * 112:t1 * 112, :].rearrange("(t p) c -> p t c", p=112))
            xn_sb = big.tile([128, 3, NPIX], f16, tag="xn")
            XNP = [(0, 112), (112, 560), (560, 1120), (1120, 1568)]
            for c0, c1 in XNP:
                nc.sync.dma_start(
                    out=xn_sb[:, :, c0:c1],
                    in_=xn_d[:].rearrange("(u p) n -> p u n", p=128)[:, :, c0:c1])
            xt_sb = big.tile([128, 16, C], f16, tag="xt")
            for q in range(4):
                nc.sync.dma_start(
                    out=xt_sb[:, q * 4:(q + 1) * 4, :],
                    in_=xt_d[q * 512:(q + 1) * 512, :].rearrange("(t p) c -> p t c", p=128))

            nc.gpsimd.load_library(library_config.local_scatter)
            ident = big.tile([128, 128], f16, tag="ident")
            masks.make_identity(nc, ident[:])

            # ---- PE warm-up: ramp the clock while ctx_t streams in ----------
            warm_ps = ps.tile([128, 128], f32, tag="ps", name="warm")
            for _ in range(NWARM):
                nc.tensor.matmul(warm_ps[:], ident[:], ident[:], start=True, stop=True)

            # ---- Tt = Wk^T @ Wq on device (warms PE with real work) ---------
            # Tt[c', c]; then G = sum_c' Tt[c', c] ctx_p[c', s] with no kf stage.
            tt_sb = big.tile([128, 3, C], f16, tag="tt")
            for cu in range(3):
                tt_ps = ps.tile([128, C], f32, tag="ps", name=f"tt{cu}")
                nc.tensor.matmul(tt_ps[:], blob_sb[:, 768 + cu * 128:768 + (cu + 1) * 128],
                                 blob_sb[:, 0:384], start=True, stop=False)
                nc.tensor.matmul(tt_ps[:], blob_sb[0:64, 1152 + cu * 128:1152 + (cu + 1) * 128],
                                 blob_sb[0:64, 384:768], start=False, stop=True)
                nc.vector.tensor_copy(tt_sb[:, cu, :], tt_ps[:])

            # ---- pooling on PE, c-major: po3u[u][c, br, bin] f32 -------------
            # One psum tile per u-chunk so accumulation groups never interleave
            # within a bank; group (u, br) = 4 consecutive mms.
            po3u = [ps.tile([128, S, 8], f32, tag="ps", name=f"po3{u}") for u in range(3)]
            for br in range(S):
                for u in range(3):
                    for k, (t, half) in enumerate(
                            ((2 * br, 0), (2 * br, 1), (2 * br + 1, 0), (2 * br + 1, 1))):
                        nc.tensor.matmul(
                            po3u[u][:, br, 0:S],
                            cxt_sb[:, t, half * C + u * 128: half * C + (u + 1) * 128],
                            ind_sb[:],
                            start=(k == 0), stop=(k == 3))
            ctxp = big.tile([128, 3, S, S], f16, tag="ctxp")
            nc.vector.tensor_copy(ctxp[:, 0], po3u[0][:, :, 0:S])
            nc.scalar.copy(ctxp[:, 1], po3u[1][:, :, 0:S])
            nc.vector.tensor_copy(ctxp[:, 2], po3u[2][:, :, 0:S])

            # -------- G = Tt^T @ ctx_p : [384, 49] f16 (single stage) --------
            g_sb = big.tile([128, 3, S * S], f16, tag="g")
            for cu in range(3):
                g_ps = ps.tile([128, S * S], f32, tag="ps")
                for v in range(3):
                    nc.tensor.matmul(g_ps[:], tt_sb[:, v, cu * 128:(cu + 1) * 128],
                                     ctxp[:, v].rearrange("p a b -> p (a b)"),
                                     start=(v == 0), stop=(v == 2))
                if cu == 1:
                    nc.scalar.copy(g_sb[:, cu, :], g_ps[:])
                else:
                    nc.vector.tensor_copy(g_sb[:, cu, :], g_ps[:])

            # ------- B1 (pair-granular) + scatter/transpose/finals pipeline -------
            expa = big.tile([S * S, NPAIR, 112], f16, tag="expa")
            d16a = big.tile([2 * W, NPAIR * NI], f16, tag="d16a")
            m_sb = big.tile([128, NPAIR * 3, 112], f16, tag="m")
            out_sb = big.tile([128, 3, NPIX], f16, tag="out")
            wwdt1 = blob_sb[0:S * S, 1536:1536 + NI]

            def logits(p):
                lg = ps.tile([S * S, 112], f32, tag="ps", name=f"lg{p}")
                for u in range(3):
                    nc.tensor.matmul(lg[:], g_sb[:, u, :],
                                     xn_sb[:, u, p * 112:(p + 1) * 112],
                                     start=(u == 0), stop=(u == 2))
                return lg

            def exp(p, lg):
                nc.scalar.activation(expa[:, p, :], lg[:],
                                     mybir.ActivationFunctionType.Exp)

            def dyn(p):
                dyn_ps = ps.tile([112, NI], f32, tag="ps", name=f"dy{p}")
                nc.tensor.matmul(dyn_ps[:], expa[:, p, :], wwdt1, start=True, stop=True)
                return dyn_ps

            def norm(p, dyn_ps):
                rec = wrk.tile([112, 1], f32, tag="rec")
                nc.vector.reciprocal(rec[:], dyn_ps[:, 25:26])
                nc.vector.tensor_scalar_mul(d16a[:, p * NI:(p + 1) * NI], dyn_ps[:], rec[:])

            def scatter(gi):
                p0, p1 = GROUPS[gi]
                npair = p1 - p0
                mt = mtp.tile([2 * W, 2 * MCOLS], f16, tag="mt", name=f"mt{gi}")
                nc.gpsimd.local_scatter(mt[:, 0:npair * MCOLS], d16a[:, p0 * NI:p1 * NI],
                                        sidx_sb[:, 0:npair * NI], channels=2 * W,
                                        num_elems=npair * MCOLS, num_idxs=npair * NI)
                return mt

            def transp(gi, mt):
                p0, p1 = GROUPS[gi]
                nch = (p1 - p0) * 3
                tp_ps = ps.tile([128, nch, 112], f16, tag="ps", name=f"tp{gi}")
                for t in range(nch):
                    nc.tensor.transpose(tp_ps[:, t, :], mt[:, t * 128:(t + 1) * 128],
                                        ident[0:112, 0:112])
                return tp_ps

            def mcopy(gi, tp_ps):
                p0, p1 = GROUPS[gi]
                nc.vector.tensor_copy(m_sb[:, p0 * 3:p1 * 3, :], tp_ps[:])

            def finals(gi):
                p0, p1 = GROUPS[gi]
                w = (p1 - p0) * 112
                for cc in range(3):
                    po = ps.tile([128, w], f32, tag="ps", name=f"fin{gi}_{cc}")
                    for pp in range(p0, p1):
                        for trel in range(3):
                            nc.tensor.matmul(po[:, (pp - p0) * 112:(pp - p0 + 1) * 112],
                                             xt_sb[:, pp + trel, cc * 128:(cc + 1) * 128],
                                             m_sb[:, pp * 3 + trel, :],
                                             start=(trel == 0), stop=(trel == 2))
                    dst = out_sb[:, cc, p0 * 112:p1 * 112]
                    if cc == 1:
                        nc.vector.tensor_copy(dst, po[:])
                    else:
                        nc.scalar.copy(dst, po[:])
                if gi in OUTDMAS:
                    ca, cb = OUTDMAS[gi]
                    nc.sync.dma_start(
                        out=out_d[:].rearrange("(u p) n -> p u n", p=128)[:, :, ca:cb],
                        in_=out_sb[:, :, ca:cb])

            # Software pipeline; program order keeps each in-order engine queue
            # from parking behind a long-latency dependence.
            def LEDN(p):
                lg = logits(p)
                exp(p, lg)
                if p >= 3:
                    norm(p - 3, dyn(p - 3))

            # Batch k issues: LEDN(2k+2), LEDN(2k+3) [-> norms SC_k needs],
            # SC_k, TP_{k-2}, MC_{k-2}, F_{k-3}. TP_g lags SC_g by two
            # batches so the in-order PE queue never parks on the Pool engine
            # while logits/dyns still need issuing; MC follows its TP
            # immediately; finals trail one more batch.
            mts, tps = {}, {}
            LEDN(0), LEDN(1), LEDN(2), LEDN(3)        # ... D0 N0
            mts[0] = scatter(0)
            LEDN(4), LEDN(5)                          # N1, N2
            mts[1] = scatter(1)
            LEDN(6), LEDN(7)                          # N3, N4
            mts[2] = scatter(2)
            tps[0] = transp(0, mts[0]); mcopy(0, tps[0])
            LEDN(8), LEDN(9)                          # N5, N6
            mts[3] = scatter(3)
            tps[1] = transp(1, mts[1]); mcopy(1, tps[1])
            finals(0)
            LEDN(10), LEDN(11)                        # N7, N8
            mts[4] = scatter(4)
            tps[2] = transp(2, mts[2]); mcopy(2, tps[2])
            finals(1)
            LEDN(12), LEDN(13)                        # N9, N10
            mts[5] = scatter(5)
            norm(11, dyn(11))
            norm(12, dyn(12))
            norm(13, dyn(13))
            tps[3] = transp(3, mts[3]); mcopy(3, tps[3])
            finals(2)
            mts[6] = scatter(6)
            tps[4] = transp(4, mts[4]); mcopy(4, tps[4])
            finals(3)
            mts[7] = scatter(7)
            tps[5] = transp(5, mts[5]); mcopy(5, tps[5])
            finals(4)
            mts[8] = scatter(8)
            tps[6] = transp(6, mts[6]); mcopy(6, tps[6])
            finals(5)
            tps[7] = transp(7, mts[7]); mcopy(7, tps[7])
            finals(6)
            tps[8] = transp(8, mts[8]); mcopy(8, tps[8])
            finals(7)
            finals(8)
    nc.finalize()
    return nc


def _static_inputs():
    # scatter index table for TWO adjacent pairs: pixel p = hl*56 + w,
    # tap j = 5*di + dj; second pair's M^T lives at col offset MCOLS.
    sidx = np.full((2 * W, 2 * NI), -1, np.int16)
    for half in range(2):
        for hl in range(2):
            for w in range(W):
                for di in range(KK):
                    for dj in range(KK):
                        sidx[hl * W + w, half * NI + 5 * di + dj] = \
                            half * MCOLS + (hl + di) * PADW + w + dj
    # pooling indicator: partition i = pixel-pair (2i, 2i+1) within a 4-row
    # chunk; bin-col = (2i % 56)//8; value 1/64 (mean over the 8x8 bin).
    ind = np.zeros((112, S), np.float16)
    for i in range(112):
        ind[i, ((2 * i) % W) // 8] = 1.0 / 64.0
    return sidx, ind


def _prep(x, ctx, Wq, Wk, Wwd):
    sidx, ind = _static_inputs()
    blob = np.zeros((128, FB), np.float16)
    blob[:, 0:384] = Wq[0:128, :]
    blob[0:64, 384:768] = Wq[128:192, :]
    blob[:, 768:1152] = Wk[0:128, :]
    blob[0:64, 1152:1536] = Wk[128:192, :]
    blob[0:S * S, 1536:1536 + NI] = np.concatenate(
        [Wwd.T, np.ones((S * S, 1), np.float32)], axis=1)
    blob[0:112, 1562:1569] = ind
    in_maps = []
    for core in range(NCORES):
        b, half = core // 2, core % 2
        r0 = half * ROWS
        xn = np.ascontiguousarray(x[b, :, r0:r0 + ROWS, :].reshape(C, NPIX)).astype(np.float16)
        xp = np.zeros((PADR, PADW, C), np.float32)
        lo, hi = max(0, r0 - 2), min(H, r0 + ROWS + 2)
        xp[lo - (r0 - 2):hi - (r0 - 2), 2:2 + W, :] = np.transpose(x[b, :, lo:hi, :], (1, 2, 0))
        xt = xp.reshape(WSPACE, C).astype(np.float16)
        cxt = np.ascontiguousarray(
            ctx[b].reshape(C, HW).T).astype(np.float16).reshape(NPP, 2 * C)
        in_maps.append(dict(cxt=cxt, blob=blob, xn=xn, sidx=sidx, xt=xt))
    return in_maps


def kernel(x, ctx, Wq, Wk, Wwd, _trace=False):
    from concourse.bass_utils import run_bass_kernel_spmd

    x, ctx = np.asarray(x), np.asarray(ctx)
    Wq, Wk, Wwd = np.asarray(Wq), np.asarray(Wk), np.asarray(Wwd)
    if "nc" not in _cached:
        _cached["nc"] = _build_nc()
    in_maps = _prep(x, ctx, Wq, Wk, Wwd)
    res = run_bass_kernel_spmd(_cached["nc"], in_maps, list(range(NCORES)), trace=_trace)
    _cached["last_result"] = res
    out = np.empty((B, C, H, W), np.float32)
    for core in range(NCORES):
        b, half = core // 2, core % 2
        r0 = half * ROWS
        out[b, :, r0:r0 + ROWS, :] = res.results[core]["out"].astype(np.float32).reshape(C, ROWS, W)
    return out


# revision 32
# speedup vs baseline: 1.0084x; 1.0084x over previous
"""ContMix kernel for TRN2, 8 NeuronCores — v3.

Sharding: (batch b, H-half) -> 8 cores. Each core computes out[b, :, r0:r0+28, :].

v3 pipeline (per core):
  PE warm-up matmuls (ident@ident) ramp the PE clock to full speed early.
  pooling on PE, c-major: lhsT = ctx_t chunk [112, 128c], rhs = ind [112, 7]
    -> po3 [128, u, br, bin] f32 accumulated 4 mms per (u, br); one DVE
    packing copy -> ctx_p [128, 3, 49]  (no transpose roundtrip)
  kf = Wk @ ctx_p ; G = Wq^T @ kf  (f16 matmuls)
  B1 pair-granular (112 cols): logits lg_p = G^T xn_p; exp; dyn_p = expa_p^T
    wwdt1; normalize -> d16a   (software-pipelined across engines)
  scatter calls of [1,2,2,2,2,2,2,1] pairs -> M^T; PE transpose -> M chunks
  finals: banded matmuls xt^T @ M per scatter-call group; out in 4 DMAs
DMA order: blob(+ind), sidx, ctx_t(8pc), xn(3pc), xt(4pc) — all f16.
"""

import numpy as np

B, C, H, W = 4, 384, 56, 56
KK, S = 5, 7
NCORES = 8
ROWS = H // 2              # 28 rows per core
NPIX = ROWS * W            # 1568
PADR = ROWS + 4            # 32 padded rows
PADW = 64                  # padded width
WSPACE = PADR * PADW       # 2048 padded pixels
NPAIR = ROWS // 2          # 14 output row-pairs
D2 = C // 2                # 192
NI = 26                    # scatter idxs (25 taps + 1 denom col)
MCOLS = 3 * 128            # 384 = w''-space per pair
HW = H * W                 # 3136
NPP = HW // 2              # 1568 pixel-pairs (full image, for pooling)
FB = 1569                  # blob: wqA 0:384, wqB 384:768, wkA 768:1152, wkB 1152:1536, wwdt1 1536:1562, ind 1562:1569
NWARM = 24                 # PE clock warm-up matmuls

# scatter-call pair grouping: starts early (1 pair), ends small (1-pair calls)
GROUPS = [(0, 1), (1, 3), (3, 5), (5, 7), (7, 9), (9, 11), (11, 12), (12, 13), (13, 14)]
# out DMA pieces fired at end of finals(g): g -> (col0, col1)
OUTDMAS = {2: (0, 560), 4: (560, 1008), 6: (1008, 1312), 8: (1312, 1568)}

_cached = {}


def _build_nc():
    import concourse.tile as tile
    from concourse import bacc, mybir, library_config, masks

    f32, f16, i16 = mybir.dt.float32, mybir.dt.float16, mybir.dt.int16
    nc = bacc.Bacc("TRN2", target_bir_lowering=False, debug=False)

    cxt_d = nc.dram_tensor("cxt", [NPP, 2 * C], f16, kind="ExternalInput")
    blob_d = nc.dram_tensor("blob", [128, FB], f16, kind="ExternalInput")
    xn_d = nc.dram_tensor("xn", [C, NPIX], f16, kind="ExternalInput")
    sidx_d = nc.dram_tensor("sidx", [2 * W, 2 * NI], i16, kind="ExternalInput")
    xt_d = nc.dram_tensor("xt", [WSPACE, C], f16, kind="ExternalInput")
    out_d = nc.dram_tensor("out", [C, NPIX], f16, kind="ExternalOutput")

    with tile.TileContext(nc) as tc:
        with (
            tc.tile_pool(name="big", bufs=1) as big,
            tc.tile_pool(name="wrk", bufs=3) as wrk,
            tc.tile_pool(name="mtp", bufs=5) as mtp,
            tc.tile_pool(name="ps", bufs=8, space="PSUM") as ps,
        ):
            # ---------------- input DMAs (SP queue = transfer order) --------------
            blob_sb = big.tile([128, FB], f16, tag="blob")
            nc.sync.dma_start(out=blob_sb[:], in_=blob_d[:])
            ind_sb = blob_sb[0:112, 1562:1569]
            sidx_sb = big.tile([2 * W, 2 * NI], i16, tag="sidx")
            nc.sync.dma_start(out=sidx_sb[:], in_=sidx_d[:])
            # ctx_t pieces: chunk t = 112 pixel-pairs = 4 image rows; bin-row br
            # = chunks 2br,2br+1.
            cxt_sb = big.tile([112, 14, 2 * C], f16, tag="cxt")
            CXP = [(0, 2), (2, 4), (4, 6), (6, 8), (8, 10), (10, 12), (12, 13), (13, 14)]
            for t0, t1 in CXP:
                nc.sync.dma_start(
                    out=cxt_sb[:, t0:t1, :],
                    in_=cxt_d[t0 * 112:t1 * 112, :].rearrange("(t p) c -> p t c", p=112))
            xn_sb = big.tile([128, 3, NPIX], f16, tag="xn")
            XNP = [(0, 112), (112, 560), (560, 1120), (1120, 1568)]
            for c0, c1 in XNP:
                nc.sync.dma_start(
                    out=xn_sb[:, :, c0:c1],
                    in_=xn_d[:].rearrange("(u p) n -> p u n", p=128)[:, :, c0:c1])
            xt_sb = big.tile([128, 16, C], f16, tag="xt")
            for q in range(4):
                nc.sync.dma_start(
                    out=xt_sb[:, q * 4:(q + 1) * 4, :],
                    in_=xt_d[q * 512:(q + 1) * 512, :].rearrange("(t p) c -> p t c", p=128))

            nc.gpsimd.load_library(library_config.local_scatter)
            ident = big.tile([128, 128], f16, tag="ident")
            masks.make_identity(nc, ident[:])

            # ---- PE warm-up: ramp the clock while ctx_t streams in ----------
            warm_ps = ps.tile([128, 128], f32, tag="ps", name="warm")
            for _ in range(NWARM):
                nc.tensor.matmul(warm_ps[:], ident[:], ident[:], start=True, stop=True)

            # ---- Tt = Wk^T @ Wq on device (warms PE with real work) ---------
            # Tt[c', c]; then G = sum_c' Tt[c', c] ctx_p[c', s] with no kf stage.
            tt_sb = big.tile([128, 3, C], f16, tag="tt")
            for cu in range(3):
                tt_ps = ps.tile([128, C], f32, tag="ps", name=f"tt{cu}")
                nc.tensor.matmul(tt_ps[:], blob_sb[:, 768 + cu * 128:768 + (cu + 1) * 128],
                                 blob_sb[:, 0:384], start=True, stop=False)
                nc.tensor.matmul(tt_ps[:], blob_sb[0:64, 1152 + cu * 128:1152 + (cu + 1) * 128],
                                 blob_sb[0:64, 384:768], start=False, stop=True)
                nc.vector.tensor_copy(tt_sb[:, cu, :], tt_ps[:])

            # ---- pooling on PE, c-major: po3u[u][c, br, bin] f32 -------------
            # One psum tile per u-chunk so accumulation groups never interleave
            # within a bank; group (u, br) = 4 consecutive mms.
            po3u = [ps.tile([128, S, 8], f32, tag="ps", name=f"po3{u}") for u in range(3)]
            for br in range(S):
                for u in range(3):
                    for k, (t, half) in enumerate(
                            ((2 * br, 0), (2 * br, 1), (2 * br + 1, 0), (2 * br + 1, 1))):
                        nc.tensor.matmul(
                            po3u[u][:, br, 0:S],
                            cxt_sb[:, t, half * C + u * 128: half * C + (u + 1) * 128],
                            ind_sb[:],
                            start=(k == 0), stop=(k == 3))
            ctxp = big.tile([128, 3, S, S], f16, tag="ctxp")
            nc.vector.tensor_copy(ctxp[:, 0], po3u[0][:, :, 0:S])
            nc.scalar.copy(ctxp[:, 1], po3u[1][:, :, 0:S])
            nc.vector.tensor_copy(ctxp[:, 2], po3u[2][:, :, 0:S])

            # -------- G = Tt^T @ ctx_p : [384, 49] f16 (single stage) --------
            g_sb = big.tile([128, 3, S * S], f16, tag="g")
            for cu in range(3):
                g_ps = ps.tile([128, S * S], f32, tag="ps")
                for v in range(3):
                    nc.tensor.matmul(g_ps[:], tt_sb[:, v, cu * 128:(cu + 1) * 128],
                                     ctxp[:, v].rearrange("p a b -> p (a b)"),
                                     start=(v == 0), stop=(v == 2))
                if cu == 1:
                    nc.scalar.copy(g_sb[:, cu, :], g_ps[:])
                else:
                    nc.vector.tensor_copy(g_sb[:, cu, :], g_ps[:])

            # ------- B1 (pair-granular) + scatter/transpose/finals pipeline -------
            expa = big.tile([S * S, NPAIR, 112], f16, tag="expa")
            d16a = big.tile([2 * W, NPAIR * NI], f16, tag="d16a")
            m_sb = big.tile([128, NPAIR * 3, 112], f16, tag="m")
            out_sb = big.tile([128, 3, NPIX], f16, tag="out")
            wwdt1 = blob_sb[0:S * S, 1536:1536 + NI]

            def logits(p):
                lg = ps.tile([S * S, 112], f32, tag="ps", name=f"lg{p}")
                for u in range(3):
                    nc.tensor.matmul(lg[:], g_sb[:, u, :],
                                     xn_sb[:, u, p * 112:(p + 1) * 112],
                                     start=(u == 0), stop=(u == 2))
                return lg

            def exp(p, lg):
                nc.scalar.activation(expa[:, p, :], lg[:],
                                     mybir.ActivationFunctionType.Exp)

            def dyn(p):
                dyn_ps = ps.tile([112, NI], f32, tag="ps", name=f"dy{p}")
                nc.tensor.matmul(dyn_ps[:], expa[:, p, :], wwdt1, start=True, stop=True)
                return dyn_ps

            def norm(p, dyn_ps):
                rec = wrk.tile([112, 1], f32, tag="rec")
                nc.vector.reciprocal(rec[:], dyn_ps[:, 25:26])
                nc.vector.tensor_scalar_mul(d16a[:, p * NI:(p + 1) * NI], dyn_ps[:], rec[:])

            def scatter(gi):
                p0, p1 = GROUPS[gi]
                npair = p1 - p0
                mt = mtp.tile([2 * W, 2 * MCOLS], f16, tag="mt", name=f"mt{gi}")
                nc.gpsimd.local_scatter(mt[:, 0:npair * MCOLS], d16a[:, p0 * NI:p1 * NI],
                                        sidx_sb[:, 0:npair * NI], channels=2 * W,
                                        num_elems=npair * MCOLS, num_idxs=npair * NI)
                return mt

            def transp(gi, mt):
                p0, p1 = GROUPS[gi]
                nch = (p1 - p0) * 3
                tp_ps = ps.tile([128, nch, 112], f16, tag="ps", name=f"tp{gi}")
                for t in range(nch):
                    nc.tensor.transpose(tp_ps[:, t, :], mt[:, t * 128:(t + 1) * 128],
                                        ident[0:112, 0:112])
                return tp_ps

            def mcopy(gi, tp_ps):
                p0, p1 = GROUPS[gi]
                nc.vector.tensor_copy(m_sb[:, p0 * 3:p1 * 3, :], tp_ps[:])

            def finals(gi):
                p0, p1 = GROUPS[gi]
                w = (p1 - p0) * 112
                for cc in range(3):
                    po = ps.tile([128, w], f32, tag="ps", name=f"fin{gi}_{cc}")
                    for pp in range(p0, p1):
                        for trel in range(3):
                            nc.tensor.matmul(po[:, (pp - p0) * 112:(pp - p0 + 1) * 112],
                                             xt_sb[:, pp + trel, cc * 128:(cc + 1) * 128],
                                             m_sb[:, pp * 3 + trel, :],
                                             start=(trel == 0), stop=(trel == 2))
                    dst = out_sb[:, cc, p0 * 112:p1 * 112]
                    if cc == 1:
                        nc.vector.tensor_copy(dst, po[:])
                    else:
                        nc.scalar.copy(dst, po[:])
                if gi in OUTDMAS:
                    ca, cb = OUTDMAS[gi]
                    nc.sync.dma_start(
                        out=out_d[:].rearrange("(u p) n -> p u n", p=128)[:, :, ca:cb],
                        in_=out_sb[:, :, ca:cb])

            # Software pipeline; program order keeps each in-order engine queue
            # from parking behind a long-latency dependence.
            def LEDN(p):
                lg = logits(p)
                exp(p, lg)
                if p >= 3:
                    norm(p - 3, dyn(p - 3))

            # Batch k issues: LEDN(2k+2), LEDN(2k+3) [-> norms SC_k needs],
            # SC_k, TP_{k-2}, MC_{k-2}, F_{k-3}. TP_g lags SC_g by two
            # batches so the in-order PE queue never parks on the Pool engine
            # while logits/dyns still need issuing; MC follows its TP
            # immediately; finals trail one more batch.
            mts, tps = {}, {}
            LEDN(0), LEDN(1), LEDN(2), LEDN(3)        # ... D0 N0
            mts[0] = scatter(0)
            LEDN(4), LEDN(5)                          # N1, N2
            mts[1] = scatter(1)
            LEDN(6), LEDN(7)                          # N3, N4
            mts[2] = scatter(2)
            tps[0] = transp(0, mts[0]); mcopy(0, tps[0])
            LEDN(8), LEDN(9)                          # N5, N6
            mts[3] = scatter(3)
            tps[1] = transp(1, mts[1]); mcopy(1, tps[1])
            finals(0)
            LEDN(10), LEDN(11)                        # N7, N8
            mts[4] = scatter(4)
            tps[2] = transp(2, mts[2]); mcopy(2, tps[2])
            finals(1)
            LEDN(12), LEDN(13)                        # N9, N10
            mts[5] = scatter(5)
            norm(11, dyn(11))
            norm(12, dyn(12))
            norm(13, dyn(13))
            tps[3] = transp(3, mts[3]); mcopy(3, tps[3])
            finals(2)
            mts[6] = scatter(6)
            tps[4] = transp(4, mts[4]); mcopy(4, tps[4])
            finals(3)
            mts[7] = scatter(7)
            tps[5] = transp(5, mts[5]); mcopy(5, tps[5])
            finals(4)
            mts[8] = scatter(8)
            tps[6] = transp(6, mts[6]); mcopy(6, tps[6])
            finals(5)
            tps[7] = transp(7, mts[7]); mcopy(7, tps[7])
            finals(6)
            tps[8] = transp(8, mts[8]); mcopy(8, tps[8])
            finals(7)
            finals(8)
    nc.finalize()
    return nc


def _static_inputs():
    # scatter index table for TWO adjacent pairs: pixel p = hl*56 + w,
    # tap j = 5*di + dj; second pair's M^T lives at col offset MCOLS.
    sidx = np.full((2 * W, 2 * NI), -1, np.int16)
    for half in range(2):
        for hl in range(2):
            for w in range(W):
                for di in range(KK):
                    for dj in range(KK):
                        sidx[hl * W + w, half * NI + 5 * di + dj] = \
                            half * MCOLS + (hl + di) * PADW + w + dj
    # pooling indicator: partition i = pixel-pair (2i, 2i+1) within a 4-row
    # chunk; bin-col = (2i % 56)//8; value 1/64 (mean over the 8x8 bin).
    ind = np.zeros((112, S), np.float16)
    for i in range(112):
        ind[i, ((2 * i) % W) // 8] = 1.0 / 64.0
    return sidx, ind


def _prep(x, ctx, Wq, Wk, Wwd):
    sidx, ind = _static_inputs()
    blob = np.zeros((128, FB), np.float16)
    blob[:, 0:384] = Wq[0:128, :]
    blob[0:64, 384:768] = Wq[128:192, :]
    blob[:, 768:1152] = Wk[0:128, :]
    blob[0:64, 1152:1536] = Wk[128:192, :]
    blob[0:S * S, 1536:1536 + NI] = np.concatenate(
        [Wwd.T, np.ones((S * S, 1), np.float32)], axis=1)
    blob[0:112, 1562:1569] = ind
    in_maps = []
    for core in range(NCORES):
        b, half = core // 2, core % 2
        r0 = half * ROWS
        xn = np.ascontiguousarray(x[b, :, r0:r0 + ROWS, :].reshape(C, NPIX)).astype(np.float16)
        xp = np.zeros((PADR, PADW, C), np.float32)
        lo, hi = max(0, r0 - 2), min(H, r0 + ROWS + 2)
        xp[lo - (r0 - 2):hi - (r0 - 2), 2:2 + W, :] = np.transpose(x[b, :, lo:hi, :], (1, 2, 0))
        xt = xp.reshape(WSPACE, C).astype(np.float16)
        cxt = np.ascontiguousarray(
            ctx[b].reshape(C, HW).T).astype(np.float16).reshape(NPP, 2 * C)
        in_maps.append(dict(cxt=cxt, blob=blob, xn=xn, sidx=sidx, xt=xt))
    return in_maps


def kernel(x, ctx, Wq, Wk, Wwd, _trace=False):
    from concourse.bass_utils import run_bass_kernel_spmd

    x, ctx = np.asarray(x), np.asarray(ctx)
    Wq, Wk, Wwd = np.asarray(Wq), np.asarray(Wk), np.asarray(Wwd)
    if "nc" not in _cached:
        _cached["nc"] = _build_nc()
    in_maps = _prep(x, ctx, Wq, Wk, Wwd)
    res = run_bass_kernel_spmd(_cached["nc"], in_maps, list(range(NCORES)), trace=_trace)
    _cached["last_result"] = res
    out = np.empty((B, C, H, W), np.float32)
    for core in range(NCORES):
        b, half = core // 2, core % 2
        r0 = half * ROWS
        out[b, :, r0:r0 + ROWS, :] = res.results[core]["out"].astype(np.float32).reshape(C, ROWS, W)
    return out
